# revision 15
# baseline (speedup 1.0000x reference)
"""2-layer GAT (PyG GATConv semantics) -> FC, output = y[root] only, on TRN2.

The reference returns y[root_idx][None, :] ([1, 64]): the final features of
the first node with x[:, 0] == 0. That value depends only on the root's
2-hop in-neighborhood: layer-2 softmax/aggregation over the root's in-edges
(plus self-loop), and layer-1 GAT outputs h1[j] for every source j of those
edges. The host extracts the ~22-node / ~500-edge-slot sub-problem (the
"shard edges by dst, gather src features" prep from the sharding hint,
specialized to the single output row) and packs per-dst edge blocks of raw
x features at a uniform padded width; the device runs all network math.
The reduced problem is far below single-core granularity, so the same
program runs replicated on all 8 cores and core 0's output is taken.

Device-efficiency structure (v2):
  - everything on-chip is fp16 (weights, features, activations) with f32
    PSUM accumulation; end-to-end error vs the f32 reference is ~5e-4.
  - single fused fp16 DRAM tensor, loaded by 4 parallel dma_starts on 4
    different engine queues, ordered so the first matmul's operands land
    first (the baseline's serialized 6-DMA load was ~5.7us; this is ~1.3us).
  - single-chunk compute: E1 <= 512 slot columns fit one PSUM bank, so
    every big matmul runs once instead of per-chunk.
  - a_src[h, e] = att1_src[h].(W1 x_src) = (att1_src[h] W1_h).x_src: the
    [4, 128] asrcW/adstW logit projections are folded from weights on the
    host (same for layer 2: a2sW2 = att2_src@W2, a2dW2 = att2_dst@W2, so
    layer-2 logits come straight from h1 and h2 is never materialized
    per-node).
  - layer-2 aggregation uses linearity: sum_j alpha_j (W2 h1_j) =
    W2 (sum_j alpha_j h1_j), so W2 is applied once to the aggregate.
  - softmax denominators divide AFTER the relu (relu(x/d) = relu(x)/d for
    d > 0), so layer 2 runs on unnormalized exp weights and a single [1,64]
    scale fixes it at the very end; the layer-2 denominator comes for free
    from the Exp activation's accum_out.
  - duplicate root in-edges are folded as a per-node multiplicity row
    multiplied into the exp weights (one DVE op that also row-sums the
    layer-2 denominator via accum_out).
  - pad slots carry a host-solved feature vector v with adstW @ v = -500
    (and zeroed src features), so padded logits underflow exp() to exactly
    0 with no mask tensor or extra ops; exp inputs are biased by -3 so all
    fp16 intermediates stay far from overflow (the softmax ratio is
    invariant).
  - the Tile kernel tail is minimized (see FastTileContext).
"""

import sys

if "/opt/trn_rl_repo" not in sys.path:
    sys.path.insert(0, "/opt/trn_rl_repo")

import numpy as np

import concourse.bacc as bacc
import concourse.mybir as mybir
import concourse.tile as tile
from concourse.bass_utils import run_bass_kernel_spmd


class FastTileContext(tile.TileContext):
    """TileContext with a minimal kernel tail.

    The stock tail emits a DMA-queue DRAIN fence (16 sub-queue fence
    descriptors at ~300ns each, ~5us serial), two all-engine barriers and a
    ~250-semaphore clear loop. Here the global-clock completion waits are
    KEPT (attached to a NOP on SP) -- every DMA including the output store
    has retired before the engines halt, which is what output validity
    requires (dropping these waits corrupts results) -- while the DRAIN
    fence, the semaphore-clear loop and the second barrier are dropped.
    Dirty end-of-run semaphore state is harmless: the framework preamble of
    every execution resets the kernel semaphore range before user code.
    """

    def _drain_and_barrier(self, tick_clock, wait_clock):
        from concourse.vector_clock import ScopedClock
        nop = self.nc.sync.nop(nofuse=True)
        wait_clock.add_sem_waits(
            nop.ins, ScopedClock({None: tick_clock.global_clock})
        )
        self.nc.all_engine_barrier(sem_only=True)
        popped = self.nc._tile_sem_poison_stack.pop()
        assert popped is self._sem_poison

F16 = mybir.dt.float16
F32 = mybir.dt.float32
AF = mybir.ActivationFunctionType
ALU = mybir.AluOpType
AX = mybir.AxisListType

NEG_SLOPE = 0.2
EXP_BIAS = -3.0  # exp(x + EXP_BIAS): keeps fp16 exp weights small; the
                 # softmax ratio (and the deferred 1/denom) is invariant


def _f16(a):
    return np.ascontiguousarray(np.asarray(a, dtype=np.float16))


def _prep(inputs):
    """Host prep: graph slicing, packing, and weight-derived constants."""
    x = np.asarray(inputs["x"], dtype=np.float32)
    ei = np.asarray(inputs["edge_index"])
    src = ei[0].astype(np.int64)
    dst = ei[1].astype(np.int64)
    W1 = np.asarray(inputs["W1"], dtype=np.float32)       # [256, 128]
    att1_src = np.asarray(inputs["att1_src"], np.float32)  # [4, 64]
    att1_dst = np.asarray(inputs["att1_dst"], np.float32)
    W2 = np.asarray(inputs["W2"], dtype=np.float32)       # [64, 256]
    att2_src = np.asarray(inputs["att2_src"], np.float32).ravel()  # [64]
    att2_dst = np.asarray(inputs["att2_dst"], np.float32).ravel()
    Wfc = np.asarray(inputs["Wfc"], dtype=np.float32)     # [64, 64]
    b1 = np.asarray(inputs["b1"], np.float32).ravel()     # [256]
    b2 = np.asarray(inputs["b2"], np.float32).ravel()     # [64]
    bfc = np.asarray(inputs["bfc"], np.float32).ravel()   # [64]

    H, HID = att1_src.shape
    IN = W1.shape[1]
    assert IN == 128 and H == 4 and HID == 64 and W2.shape == (64, 256)

    asrcW = np.stack([att1_src[h] @ W1[h * HID:(h + 1) * HID] for h in range(H)])
    adstW = np.stack([att1_dst[h] @ W1[h * HID:(h + 1) * HID] for h in range(H)])
    # pad-slot dst feature: adstW @ v = -500 for every head (least-norm).
    # -500 -> lrelu -> -100 -> exp underflows to 0; v stays small enough
    # that fp16 rounding moves the logit by well under 1.
    v_mask = np.linalg.lstsq(adstW.astype(np.float64),
                             np.full(H, -500.0), rcond=None)[0]
    v16 = v_mask.astype(np.float16)
    assert np.abs(adstW.astype(np.float64) @ v16.astype(np.float64) + 500).max() < 50

    a2sW2 = att2_src @ W2  # [256]
    a2dW2 = att2_dst @ W2

    # ---- root + 2-hop neighborhood
    root = int(np.argmax(x[:, 0] == 0.0))
    r_srcs = src[dst == root]
    L1 = np.unique(np.concatenate([r_srcs, np.array([root], np.int64)]))
    n1 = int(L1.size)
    mult = np.bincount(np.searchsorted(L1, r_srcs), minlength=n1).astype(np.float64)
    mult[np.searchsorted(L1, root)] += 1.0  # appended self-loop
    root_blk = int(np.searchsorted(L1, root))

    sel = np.isin(dst, L1)
    e_src = src[sel]
    d_idx = np.searchsorted(L1, dst[sel])
    cnt = np.bincount(d_idx, minlength=n1)   # real in-degree per L1 node

    D = int(cnt.max() + 1)                   # uniform block width (+self-loop)
    E1 = n1 * D
    assert E1 <= 512, f"padded slot count {E1} exceeds single-bank design"

    order = np.argsort(d_idx, kind="stable")
    starts = np.zeros(n1, np.int64)
    starts[1:] = np.cumsum(cnt)[:-1]
    within = np.arange(d_idx.size) - starts[d_idx[order]]
    srcflat = np.full(E1, -1, np.int64)
    srcflat[d_idx[order] * D + within] = e_src[order]
    srcflat[np.arange(n1) * D + cnt] = L1    # self-loop slot per node
    valid = srcflat >= 0

    XE = np.zeros((E1, IN), np.float16)
    XE[valid] = x[srcflat[valid]].astype(np.float16)
    XD = np.repeat(x[L1].astype(np.float16), D, axis=0)
    XD[~valid] = v16

    # ---- fused fp16 input tensor: [ asrcT|adstT | XET | XDT | consts ]
    off = {}
    cols = []

    def put(name, arr, rows=128):
        a = np.zeros((128, arr.shape[1]), np.float16)
        a[:rows] = _f16(arr)
        off[name] = sum(c.shape[1] for c in cols)
        cols.append(a)

    p = np.arange(128)
    SEL_lo = (p[None, :] // HID == np.arange(H)[:, None]).astype(np.float32)
    SEL_hi = (p[None, :] // HID + 2 == np.arange(H)[:, None]).astype(np.float32)

    put("xet", XE.T)                                         # [128, E1]
    put("xdt", XD.T)                                         # [128, E1]
    put("ch", np.concatenate([asrcW.T, adstW.T], axis=1))    # [128, 8]
    put("w1t", W1.T)                                         # [128, 256]
    put("sel_lo", SEL_lo, rows=4)                            # [4, 128]
    put("sel_hi", SEL_hi, rows=4)
    put("a2lo", np.stack([a2sW2[:128], a2dW2[:128]], 1))     # [128, 2] s|d
    put("a2hi", np.stack([a2sW2[128:], a2dW2[128:]], 1))
    put("w2lo", W2.T[:128])                                  # [128, 64]
    put("w2hi", W2.T[128:])
    put("wfct", Wfc.T, rows=64)                              # [64, 64]
    put("mlt", mult[None, :], rows=1)                        # [1, n1]
    put("b1c", b1.reshape(2, 128).T)                         # [128, 2]
    put("b2c", b2[:, None], rows=64)                         # [64, 1]
    put("bfcr", bfc[None, :], rows=1)                        # [1, 64]
    put("eb", np.full((4, 1), EXP_BIAS), rows=4)             # [4, 1]

    c2_off = off["ch"] + 8
    for k in off:
        if k not in ("xet", "xdt", "ch"):
            off[k] -= c2_off

    xall = np.ascontiguousarray(np.concatenate(cols, axis=1))
    return dict(
        n1=n1, D=D, E1=E1, root_blk=root_blk, off=off, xall=xall,
        wc=xall.shape[1] - c2_off,
        b1_zero=bool(np.all(b1 == 0.0)), b2_zero=bool(np.all(b2 == 0.0)),
        bfc_zero=bool(np.all(bfc == 0.0)),
    )


def _build_nc(n1, D, E1, root_blk, off, WT, WC, b1_zero, b2_zero,
              bfc_zero):
    nc = bacc.Bacc(None, target_bir_lowering=False, debug=False)
    xall_d = nc.dram_tensor("xall", [128, WT], F16, kind="ExternalInput")
    out_d = nc.dram_tensor("out", [1, 64], F32, kind="ExternalOutput")

    with FastTileContext(nc) as tc:
        with (
            tc.tile_pool(name="sb", bufs=1) as sb,
            tc.tile_pool(name="psb", bufs=1, space="PSUM") as psb,
            tc.tile_pool(name="pss", bufs=4, space="PSUM") as pss,
        ):
            xet = sb.tile([128, E1], F16)
            xdt = sb.tile([128, E1], F16)
            c2 = sb.tile([128, WC], F16)
            ch = sb.tile([128, 8], F16)
            nc.gpsimd.dma_start(out=ch[:], in_=xall_d[:, 2 * E1:2 * E1 + 8])
            nc.sync.dma_start(out=xet[:], in_=xall_d[:, 0:E1])
            nc.scalar.dma_start(out=xdt[:], in_=xall_d[:, E1:2 * E1])
            nc.gpsimd.dma_start(out=c2[:], in_=xall_d[:, 2 * E1 + 8:
                                                      2 * E1 + 8 + WC])
            ones = sb.tile([1, 128], F16)
            nc.vector.memset(ones[:], 1.0)

            def C(name, p, w):
                o = off[name]
                return c2[0:p, o:o + w]

            # ---- layer 1: attention logits -> exp weights
            p_e = psb.tile([4, E1], F32, tag="a")
            nc.tensor.matmul(p_e[:], ch[:, 0:4], xet[:], start=True, stop=False)
            nc.tensor.matmul(p_e[:], ch[:, 4:8], xdt[:], start=False, stop=True)
            e_sb = sb.tile([4, E1], F16)
            exf = sb.tile([4, E1], F16)
            with tc.high_priority():
                nc.scalar.activation(out=e_sb[:], in_=p_e[:], func=AF.Prelu,
                                     alpha=NEG_SLOPE)
                nc.scalar.activation(out=exf[:], in_=e_sb[:], func=AF.Exp,
                                     bias=C("eb", 4, 1))

            # ---- projected src features (PE) -> SBUF fp16 via ACT copies
            p_ht_lo = psb.tile([128, E1], F32, tag="ht_lo")
            p_ht_hi = psb.tile([128, E1], F32, tag="ht_hi")
            nc.tensor.matmul(p_ht_lo[:], C("w1t", 128, 128), xet[:])
            nc.tensor.matmul(p_ht_hi[:], c2[0:128, off["w1t"] + 128:
                                            off["w1t"] + 256], xet[:])
            # ---- per-head exp weights broadcast to feature partitions
            p_xb_lo = psb.tile([128, E1], F32, tag="a")
            p_xb_hi = psb.tile([128, E1], F32, tag="b")
            nc.tensor.matmul(p_xb_lo[:], C("sel_lo", 4, 128), exf[:])
            nc.tensor.matmul(p_xb_hi[:], C("sel_hi", 4, 128), exf[:])
            xb_lo = sb.tile([128, E1], F16)
            xb_hi = sb.tile([128, E1], F16)
            nc.scalar.copy(out=xb_lo[:], in_=p_xb_lo[:])
            nc.scalar.copy(out=xb_hi[:], in_=p_xb_hi[:])

            # ---- segment softmax denominators (uniform width D)
            denom = sb.tile([4, n1], F16)
            dinv = sb.tile([4, n1], F16)
            with nc.allow_low_precision("fp16 softmax stats, error ~5e-4"):
                nc.vector.reduce_sum(
                    out=denom[:], in_=exf[:].rearrange("p (a b) -> p a b", b=D),
                    axis=AX.X)
                nc.vector.reciprocal(out=dinv[:], in_=denom[:])

            # ---- weighted segment sums + normalize + relu -> h1 (fp16)
            w_lo = sb.tile([128, E1], F16)
            w_hi = sb.tile([128, E1], F16)
            nc.vector.tensor_mul(out=w_lo[:], in0=xb_lo[:], in1=p_ht_lo[:])
            nc.vector.tensor_mul(out=w_hi[:], in0=xb_hi[:], in1=p_ht_hi[:])
            p_dv_lo = pss.tile([128, n1], F32, tag="pss")
            p_dv_hi = pss.tile([128, n1], F32, tag="pss")
            nc.tensor.matmul(p_dv_lo[:], C("sel_lo", 4, 128), dinv[:])
            nc.tensor.matmul(p_dv_hi[:], C("sel_hi", 4, 128), dinv[:])
            h1 = {}
            for half, wt, p_dv, eng in (("lo", w_lo, p_dv_lo, nc.vector),
                                        ("hi", w_hi, p_dv_hi, nc.vector)):
                s_p = sb.tile([128, n1], F16, tag=f"s_{half}")
                with nc.allow_low_precision("fp16 segment sums, error ~5e-4"):
                    nc.vector.reduce_sum(
                        out=s_p[:], in_=wt[:].rearrange("p (a b) -> p a b", b=D),
                        axis=AX.X)
                h1t = sb.tile([128, n1], F16, tag=f"h1_{half}")
                # relu(s)*dinv == relu(s*dinv) for dinv > 0; b1 is zero here
                with nc.allow_low_precision("fp16 h1"):
                    nc.vector.scalar_tensor_tensor(
                        out=h1t[:], in0=s_p[:], scalar=0.0, in1=p_dv[:],
                        op0=ALU.max, op1=ALU.mult)
                if not b1_zero:
                    bcol = (C("b1c", 128, 1) if half == "lo"
                            else c2[0:128, off["b1c"] + 1:off["b1c"] + 2])
                    nc.vector.tensor_scalar(out=h1t[:], in0=h1t[:],
                                            scalar1=bcol, scalar2=0.0,
                                            op0=ALU.add, op1=ALU.max)
                h1[half] = h1t

            # ---- layer 2 (1 head): logits straight from h1 via folded a2W2
            p2s = pss.tile([1, n1], F32, tag="pss")
            nc.tensor.matmul(p2s[:], c2[0:128, off["a2lo"]:off["a2lo"] + 1],
                             h1["lo"][:], start=True, stop=False)
            nc.tensor.matmul(p2s[:], c2[0:128, off["a2hi"]:off["a2hi"] + 1],
                             h1["hi"][:], start=False, stop=False)
            nc.tensor.matmul(p2s[:], c2[0:128, off["a2lo"] + 1:off["a2lo"] + 2],
                             h1["lo"][:, root_blk:root_blk + 1].broadcast_to(
                                 [128, n1]), start=False, stop=False)
            nc.tensor.matmul(p2s[:], c2[0:128, off["a2hi"] + 1:off["a2hi"] + 2],
                             h1["hi"][:, root_blk:root_blk + 1].broadcast_to(
                                 [128, n1]), start=False, stop=True)
            lr2 = sb.tile([1, n1], F32)
            ex2 = sb.tile([1, n1], F16)
            w2r = sb.tile([1, n1], F16)
            den2 = sb.tile([1, 1], F32)
            d2i = sb.tile([1, 1], F32)
            with tc.high_priority():
                nc.scalar.activation(out=lr2[:], in_=p2s[:], func=AF.Prelu,
                                     alpha=NEG_SLOPE)
                nc.scalar.activation(out=ex2[:], in_=lr2[:], func=AF.Exp,
                                     bias=C("eb", 1, 1))
                with nc.allow_low_precision("fp16 softmax weights"):
                    nc.vector.scalar_tensor_tensor(
                        out=w2r[:], in0=ex2[:], scalar=1.0, in1=C("mlt", 1, n1),
                        op0=ALU.mult, op1=ALU.mult, accum_out=den2[:])
                nc.vector.reciprocal(out=d2i[:], in_=den2[:])
                if not b2_zero:
                    nc.vector.tensor_scalar_mul(out=w2r[:], in0=w2r[:],
                                                scalar1=d2i[:])

                # aggregate in h1 space (linearity), then W2 once
                p_wb = pss.tile([128, n1], F32, tag="pss")
                nc.tensor.matmul(p_wb[:], ones[:], w2r[:])
                scr = sb.tile([128, n1], F16)
                agg_lo = sb.tile([128, 1], F16)
                agg_hi = sb.tile([128, 1], F16)
                with nc.allow_low_precision("fp16 aggregate, rescaled exp"):
                    nc.vector.scalar_tensor_tensor(
                        out=scr[:], in0=h1["lo"][:], scalar=1.0, in1=p_wb[:],
                        op0=ALU.mult, op1=ALU.mult, accum_out=agg_lo[:])
                    nc.vector.scalar_tensor_tensor(
                        out=scr[:], in0=h1["hi"][:], scalar=1.0, in1=p_wb[:],
                        op0=ALU.mult, op1=ALU.mult, accum_out=agg_hi[:])
                p_h2 = pss.tile([64, 1], F32, tag="pss")
                nc.tensor.matmul(p_h2[:], C("w2lo", 128, 64), agg_lo[:],
                                 start=True, stop=False)
                nc.tensor.matmul(p_h2[:], C("w2hi", 128, 64), agg_hi[:],
                                 start=False, stop=True)
                h2v = sb.tile([64, 1], F16)
                with nc.allow_low_precision("fp16 h2"):
                    nc.vector.tensor_scalar(out=h2v[:], in0=p_h2[:],
                                            scalar1=(0.0 if b2_zero
                                                     else C("b2c", 64, 1)),
                                            scalar2=0.0,
                                            op0=ALU.add, op1=ALU.max)

                p_y = pss.tile([1, 64], F32, tag="pss")
                nc.tensor.matmul(p_y[:], h2v[:], C("wfct", 64, 64))
                y_sb = sb.tile([1, 64], F32)
                if b2_zero:
                    # h2 was aggregated with unnormalized exp weights;
                    # relu(x/d) = relu(x)/d for d > 0, so scale once here.
                    nc.vector.tensor_scalar_mul(out=y_sb[:], in0=p_y[:],
                                                scalar1=d2i[:])
                else:
                    nc.vector.tensor_scalar_mul(out=y_sb[:], in0=p_y[:],
                                                scalar1=1.0)
                if not bfc_zero:
                    nc.vector.tensor_add(out=y_sb[:], in0=y_sb[:],
                                         in1=C("bfcr", 1, 64))
                nc.sync.dma_start(out=out_d[:], in_=y_sb[:], single_packet=True)

    nc.compile()
    return nc


def kernel(**inputs):
    g = _prep(inputs)
    nc = _build_nc(g["n1"], g["D"], g["E1"], g["root_blk"], g["off"],
                   g["xall"].shape[1], g["wc"], g["b1_zero"], g["b2_zero"],
                   g["bfc_zero"])
    feed = {"xall": g["xall"]}
    res = run_bass_kernel_spmd(nc, [feed] * 8, core_ids=list(range(8)))
    return np.ascontiguousarray(res.results[0]["out"])


# revision 17
# speedup vs baseline: 1.1250x; 1.1250x over previous
"""2-layer GAT (PyG GATConv semantics) -> FC, output = y[root] only, on TRN2.

The reference returns y[root_idx][None, :] ([1, 64]): the final features of
the first node with x[:, 0] == 0. That value depends only on the root's
2-hop in-neighborhood: layer-2 softmax/aggregation over the root's in-edges
(plus self-loop), and layer-1 GAT outputs h1[j] for every source j of those
edges. The host extracts the ~22-node / ~500-edge-slot sub-problem (the
"shard edges by dst, gather src features" prep from the sharding hint,
specialized to the single output row) and packs per-dst edge blocks of raw
x features at a uniform padded width; the device runs all network math.
The reduced problem is far below single-core granularity, so the same
program runs replicated on all 8 cores and core 0's output is taken.

Device-efficiency structure (v2):
  - everything on-chip is fp16 (weights, features, activations) with f32
    PSUM accumulation; end-to-end error vs the f32 reference is ~5e-4.
  - single fused fp16 DRAM tensor, loaded by 4 parallel dma_starts on 4
    different engine queues, ordered so the first matmul's operands land
    first (the baseline's serialized 6-DMA load was ~5.7us; this is ~1.3us).
  - single-chunk compute: E1 <= 512 slot columns fit one PSUM bank, so
    every big matmul runs once instead of per-chunk.
  - a_src[h, e] = att1_src[h].(W1 x_src) = (att1_src[h] W1_h).x_src: the
    [4, 128] asrcW/adstW logit projections are folded from weights on the
    host (same for layer 2: a2sW2 = att2_src@W2, a2dW2 = att2_dst@W2, so
    layer-2 logits come straight from h1 and h2 is never materialized
    per-node).
  - layer-2 aggregation uses linearity: sum_j alpha_j (W2 h1_j) =
    W2 (sum_j alpha_j h1_j), so W2 is applied once to the aggregate.
  - softmax denominators divide AFTER the relu (relu(x/d) = relu(x)/d for
    d > 0), so layer 2 runs on unnormalized exp weights and a single [1,64]
    scale fixes it at the very end; the layer-2 denominator comes for free
    from the Exp activation's accum_out.
  - duplicate root in-edges are folded as a per-node multiplicity row
    multiplied into the exp weights (one DVE op that also row-sums the
    layer-2 denominator via accum_out).
  - pad slots carry a host-solved feature vector v with adstW @ v = -500
    (and zeroed src features), so padded logits underflow exp() to exactly
    0 with no mask tensor or extra ops; exp inputs are biased by -3 so all
    fp16 intermediates stay far from overflow (the softmax ratio is
    invariant).
  - the Tile kernel tail is minimized (see FastTileContext).
"""

import sys

if "/opt/trn_rl_repo" not in sys.path:
    sys.path.insert(0, "/opt/trn_rl_repo")

import numpy as np

import concourse.bacc as bacc
import concourse.mybir as mybir
import concourse.tile as tile
from concourse.bass_utils import run_bass_kernel_spmd


class FastTileContext(tile.TileContext):
    """TileContext with a minimal kernel tail.

    The stock tail emits a DMA-queue DRAIN fence (16 sub-queue fence
    descriptors at ~300ns each, ~5us serial), two all-engine barriers and a
    ~250-semaphore clear loop. Here the global-clock completion waits are
    KEPT (attached to a NOP on SP) -- every DMA including the output store
    has retired before the engines halt, which is what output validity
    requires (dropping these waits corrupts results) -- while the DRAIN
    fence, the semaphore-clear loop and the second barrier are dropped.
    Dirty end-of-run semaphore state is harmless: the framework preamble of
    every execution resets the kernel semaphore range before user code.
    """

    def _drain_and_barrier(self, tick_clock, wait_clock):
        from concourse.vector_clock import ScopedClock
        nop = self.nc.sync.nop(nofuse=True)
        wait_clock.add_sem_waits(
            nop.ins, ScopedClock({None: tick_clock.global_clock})
        )
        self.nc.all_engine_barrier(sem_only=True)
        popped = self.nc._tile_sem_poison_stack.pop()
        assert popped is self._sem_poison

F16 = mybir.dt.float16
F32 = mybir.dt.float32
AF = mybir.ActivationFunctionType
ALU = mybir.AluOpType
AX = mybir.AxisListType

NEG_SLOPE = 0.2
EXP_BIAS = -3.0  # exp(x + EXP_BIAS): keeps fp16 exp weights small; the
                 # softmax ratio (and the deferred 1/denom) is invariant


def _f16(a):
    return np.ascontiguousarray(np.asarray(a, dtype=np.float16))


def _prep(inputs):
    """Host prep: graph slicing, packing, and weight-derived constants."""
    x = np.asarray(inputs["x"], dtype=np.float32)
    ei = np.asarray(inputs["edge_index"])
    src = ei[0].astype(np.int64)
    dst = ei[1].astype(np.int64)
    W1 = np.asarray(inputs["W1"], dtype=np.float32)       # [256, 128]
    att1_src = np.asarray(inputs["att1_src"], np.float32)  # [4, 64]
    att1_dst = np.asarray(inputs["att1_dst"], np.float32)
    W2 = np.asarray(inputs["W2"], dtype=np.float32)       # [64, 256]
    att2_src = np.asarray(inputs["att2_src"], np.float32).ravel()  # [64]
    att2_dst = np.asarray(inputs["att2_dst"], np.float32).ravel()
    Wfc = np.asarray(inputs["Wfc"], dtype=np.float32)     # [64, 64]
    b1 = np.asarray(inputs["b1"], np.float32).ravel()     # [256]
    b2 = np.asarray(inputs["b2"], np.float32).ravel()     # [64]
    bfc = np.asarray(inputs["bfc"], np.float32).ravel()   # [64]

    H, HID = att1_src.shape
    IN = W1.shape[1]
    assert IN == 128 and H == 4 and HID == 64 and W2.shape == (64, 256)

    asrcW = np.stack([att1_src[h] @ W1[h * HID:(h + 1) * HID] for h in range(H)])
    adstW = np.stack([att1_dst[h] @ W1[h * HID:(h + 1) * HID] for h in range(H)])
    # pad-slot dst feature: adstW @ v = -500 for every head (least-norm).
    # -500 -> lrelu -> -100 -> exp underflows to 0; v stays small enough
    # that fp16 rounding moves the logit by well under 1.
    v_mask = np.linalg.lstsq(adstW.astype(np.float64),
                             np.full(H, -500.0), rcond=None)[0]
    v16 = v_mask.astype(np.float16)
    assert np.abs(adstW.astype(np.float64) @ v16.astype(np.float64) + 500).max() < 50

    a2sW2 = att2_src @ W2  # [256]
    a2dW2 = att2_dst @ W2

    # ---- root + 2-hop neighborhood
    root = int(np.argmax(x[:, 0] == 0.0))
    r_srcs = src[dst == root]
    L1 = np.unique(np.concatenate([r_srcs, np.array([root], np.int64)]))
    n1 = int(L1.size)
    mult = np.bincount(np.searchsorted(L1, r_srcs), minlength=n1).astype(np.float64)
    mult[np.searchsorted(L1, root)] += 1.0  # appended self-loop
    root_blk = int(np.searchsorted(L1, root))

    sel = np.isin(dst, L1)
    e_src = src[sel]
    d_idx = np.searchsorted(L1, dst[sel])
    cnt = np.bincount(d_idx, minlength=n1)   # real in-degree per L1 node

    D = int(cnt.max() + 1)                   # uniform block width (+self-loop)
    E1 = n1 * D
    assert E1 <= 512, f"padded slot count {E1} exceeds single-bank design"

    order = np.argsort(d_idx, kind="stable")
    starts = np.zeros(n1, np.int64)
    starts[1:] = np.cumsum(cnt)[:-1]
    within = np.arange(d_idx.size) - starts[d_idx[order]]
    srcflat = np.full(E1, -1, np.int64)
    srcflat[d_idx[order] * D + within] = e_src[order]
    srcflat[np.arange(n1) * D + cnt] = L1    # self-loop slot per node
    valid = srcflat >= 0

    XE = np.zeros((E1, IN), np.float16)
    XE[valid] = x[srcflat[valid]].astype(np.float16)
    XD = np.repeat(x[L1].astype(np.float16), D, axis=0)
    XD[~valid] = v16

    # ---- fused fp16 input tensor: [ asrcT|adstT | XET | XDT | consts ]
    off = {}
    cols = []

    def put(name, arr, rows=128):
        a = np.zeros((128, arr.shape[1]), np.float16)
        a[:rows] = _f16(arr)
        off[name] = sum(c.shape[1] for c in cols)
        cols.append(a)

    p = np.arange(128)
    SEL_lo = (p[None, :] // HID == np.arange(H)[:, None]).astype(np.float32)
    SEL_hi = (p[None, :] // HID + 2 == np.arange(H)[:, None]).astype(np.float32)

    put("asrc", asrcW.T)                                     # [128, 4]
    put("xet", XE.T)                                         # [128, E1]
    put("adst", adstW.T)                                     # [128, 4]
    put("xdt", XD.T)                                         # [128, E1]
    put("w1t", W1.T)                                         # [128, 256]
    put("sel_lo", SEL_lo, rows=4)                            # [4, 128]
    put("sel_hi", SEL_hi, rows=4)
    put("a2lo", np.stack([a2sW2[:128], a2dW2[:128]], 1))     # [128, 2] s|d
    put("a2hi", np.stack([a2sW2[128:], a2dW2[128:]], 1))
    put("w2lo", W2.T[:128])                                  # [128, 64]
    put("w2hi", W2.T[128:])
    put("wfct", Wfc.T, rows=64)                              # [64, 64]
    put("mlt", mult[None, :], rows=1)                        # [1, n1]
    put("b1c", b1.reshape(2, 128).T)                         # [128, 2]
    put("b2c", b2[:, None], rows=64)                         # [64, 1]
    put("bfcr", bfc[None, :], rows=1)                        # [1, 64]
    put("eb", np.full((4, 1), EXP_BIAS), rows=4)             # [4, 1]

    c2_off = off["w1t"]
    for k in off:
        if k not in ("asrc", "xet", "adst", "xdt"):
            off[k] -= c2_off

    xall = np.ascontiguousarray(np.concatenate(cols, axis=1))
    return dict(
        n1=n1, D=D, E1=E1, root_blk=root_blk, off=off, xall=xall,
        wc=xall.shape[1] - c2_off,
        b1_zero=bool(np.all(b1 == 0.0)), b2_zero=bool(np.all(b2 == 0.0)),
        bfc_zero=bool(np.all(bfc == 0.0)),
    )


def _build_nc(n1, D, E1, root_blk, off, WT, WC, b1_zero, b2_zero,
              bfc_zero):
    nc = bacc.Bacc(None, target_bir_lowering=False, debug=False)
    xall_d = nc.dram_tensor("xall", [128, WT], F16, kind="ExternalInput")
    out_d = nc.dram_tensor("out", [1, 64], F32, kind="ExternalOutput")

    with FastTileContext(nc) as tc:
        with (
            tc.tile_pool(name="sb", bufs=1) as sb,
            tc.tile_pool(name="psb", bufs=1, space="PSUM") as psb,
            tc.tile_pool(name="pss", bufs=4, space="PSUM") as pss,
        ):
            xet = sb.tile([128, 4 + E1], F16)
            xdt = sb.tile([128, 4 + E1], F16)
            c2 = sb.tile([128, WC], F16)
            nc.gpsimd.dma_start(out=c2[:], in_=xall_d[:, 2 * E1 + 8:
                                                      2 * E1 + 8 + WC])
            nc.sync.dma_start(out=xet[:], in_=xall_d[:, 0:4 + E1])
            nc.scalar.dma_start(out=xdt[:], in_=xall_d[:, 4 + E1:8 + 2 * E1])
            ones = sb.tile([1, 128], F16)
            nc.vector.memset(ones[:], 1.0)

            def C(name, p, w):
                o = off[name]
                return c2[0:p, o:o + w]

            # ---- layer 1: attention logits -> exp weights
            p_e = psb.tile([4, E1], F32, tag="a")
            nc.tensor.matmul(p_e[:], xet[:, 0:4], xet[:, 4:], start=True, stop=False)
            nc.tensor.matmul(p_e[:], xdt[:, 0:4], xdt[:, 4:], start=False, stop=True)
            e_sb = sb.tile([4, E1], F16)
            exf = sb.tile([4, E1], F16)
            with tc.high_priority():
                nc.scalar.activation(out=e_sb[:], in_=p_e[:], func=AF.Prelu,
                                     alpha=NEG_SLOPE)
                nc.scalar.activation(out=exf[:], in_=e_sb[:], func=AF.Exp,
                                     bias=C("eb", 4, 1))

            # ---- projected src features (PE) -> SBUF fp16 via ACT copies
            p_ht_lo = psb.tile([128, E1], F32, tag="ht_lo")
            p_ht_hi = psb.tile([128, E1], F32, tag="ht_hi")
            nc.tensor.matmul(p_ht_lo[:], C("w1t", 128, 128), xet[:, 4:])
            nc.tensor.matmul(p_ht_hi[:], c2[0:128, off["w1t"] + 128:
                                            off["w1t"] + 256], xet[:, 4:])
            # ---- per-head exp weights broadcast to feature partitions
            p_xb_lo = psb.tile([128, E1], F32, tag="a")
            p_xb_hi = psb.tile([128, E1], F32, tag="b")
            nc.tensor.matmul(p_xb_lo[:], C("sel_lo", 4, 128), exf[:])
            nc.tensor.matmul(p_xb_hi[:], C("sel_hi", 4, 128), exf[:])
            ht_lo = sb.tile([128, E1], F16)
            ht_hi = sb.tile([128, E1], F16)
            with tc.high_priority(offset=-1000):
                nc.scalar.copy(out=ht_lo[:], in_=p_ht_lo[:])
                nc.scalar.copy(out=ht_hi[:], in_=p_ht_hi[:])

            # ---- segment softmax denominators (uniform width D)
            denom = sb.tile([4, n1], F16)
            dinv = sb.tile([4, n1], F16)
            with nc.allow_low_precision("fp16 softmax stats, error ~5e-4"):
                nc.vector.reduce_sum(
                    out=denom[:], in_=exf[:].rearrange("p (a b) -> p a b", b=D),
                    axis=AX.X)
                nc.vector.reciprocal(out=dinv[:], in_=denom[:])

            # ---- weighted segment sums + normalize + relu -> h1 (fp16)
            w_lo = sb.tile([128, E1], F16)
            w_hi = sb.tile([128, E1], F16)
            nc.vector.tensor_mul(out=w_lo[:], in0=ht_lo[:], in1=p_xb_lo[:])
            nc.vector.tensor_mul(out=w_hi[:], in0=ht_hi[:], in1=p_xb_hi[:])
            p_dv_lo = pss.tile([128, n1], F32, tag="pss")
            p_dv_hi = pss.tile([128, n1], F32, tag="pss")
            nc.tensor.matmul(p_dv_lo[:], C("sel_lo", 4, 128), dinv[:])
            nc.tensor.matmul(p_dv_hi[:], C("sel_hi", 4, 128), dinv[:])
            h1 = {}
            for half, wt, p_dv, eng in (("lo", w_lo, p_dv_lo, nc.vector),
                                        ("hi", w_hi, p_dv_hi, nc.vector)):
                s_p = sb.tile([128, n1], F16, tag=f"s_{half}")
                with nc.allow_low_precision("fp16 segment sums, error ~5e-4"):
                    nc.vector.reduce_sum(
                        out=s_p[:], in_=wt[:].rearrange("p (a b) -> p a b", b=D),
                        axis=AX.X)
                h1t = sb.tile([128, n1], F16, tag=f"h1_{half}")
                # relu(s)*dinv == relu(s*dinv) for dinv > 0; b1 is zero here
                with nc.allow_low_precision("fp16 h1"):
                    nc.vector.scalar_tensor_tensor(
                        out=h1t[:], in0=s_p[:], scalar=0.0, in1=p_dv[:],
                        op0=ALU.max, op1=ALU.mult)
                if not b1_zero:
                    bcol = (C("b1c", 128, 1) if half == "lo"
                            else c2[0:128, off["b1c"] + 1:off["b1c"] + 2])
                    nc.vector.tensor_scalar(out=h1t[:], in0=h1t[:],
                                            scalar1=bcol, scalar2=0.0,
                                            op0=ALU.add, op1=ALU.max)
                h1[half] = h1t

            # ---- layer 2 (1 head): logits straight from h1 via folded a2W2
            p2s = pss.tile([1, n1], F32, tag="pss")
            nc.tensor.matmul(p2s[:], c2[0:128, off["a2lo"]:off["a2lo"] + 1],
                             h1["lo"][:], start=True, stop=False)
            nc.tensor.matmul(p2s[:], c2[0:128, off["a2hi"]:off["a2hi"] + 1],
                             h1["hi"][:], start=False, stop=False)
            nc.tensor.matmul(p2s[:], c2[0:128, off["a2lo"] + 1:off["a2lo"] + 2],
                             h1["lo"][:, root_blk:root_blk + 1].broadcast_to(
                                 [128, n1]), start=False, stop=False)
            nc.tensor.matmul(p2s[:], c2[0:128, off["a2hi"] + 1:off["a2hi"] + 2],
                             h1["hi"][:, root_blk:root_blk + 1].broadcast_to(
                                 [128, n1]), start=False, stop=True)
            lr2 = sb.tile([1, n1], F32)
            ex2 = sb.tile([1, n1], F16)
            w2r = sb.tile([1, n1], F16)
            den2 = sb.tile([1, 1], F32)
            d2i = sb.tile([1, 1], F32)
            with tc.high_priority():
                nc.scalar.activation(out=lr2[:], in_=p2s[:], func=AF.Prelu,
                                     alpha=NEG_SLOPE)
                nc.scalar.activation(out=ex2[:], in_=lr2[:], func=AF.Exp,
                                     bias=C("eb", 1, 1))
                with nc.allow_low_precision("fp16 softmax weights"):
                    nc.vector.scalar_tensor_tensor(
                        out=w2r[:], in0=ex2[:], scalar=1.0, in1=C("mlt", 1, n1),
                        op0=ALU.mult, op1=ALU.mult, accum_out=den2[:])
                nc.vector.reciprocal(out=d2i[:], in_=den2[:])
                if not b2_zero:
                    nc.vector.tensor_scalar_mul(out=w2r[:], in0=w2r[:],
                                                scalar1=d2i[:])

                # aggregate in h1 space (linearity), then W2 once
                p_wb = pss.tile([128, n1], F32, tag="pss")
                nc.tensor.matmul(p_wb[:], ones[:], w2r[:])
                scr = sb.tile([128, n1], F16)
                agg_lo = sb.tile([128, 1], F16)
                agg_hi = sb.tile([128, 1], F16)
                with nc.allow_low_precision("fp16 aggregate, rescaled exp"):
                    nc.vector.scalar_tensor_tensor(
                        out=scr[:], in0=h1["lo"][:], scalar=1.0, in1=p_wb[:],
                        op0=ALU.mult, op1=ALU.mult, accum_out=agg_lo[:])
                    nc.vector.scalar_tensor_tensor(
                        out=scr[:], in0=h1["hi"][:], scalar=1.0, in1=p_wb[:],
                        op0=ALU.mult, op1=ALU.mult, accum_out=agg_hi[:])
                p_h2 = pss.tile([64, 1], F32, tag="pss")
                nc.tensor.matmul(p_h2[:], C("w2lo", 128, 64), agg_lo[:],
                                 start=True, stop=False)
                nc.tensor.matmul(p_h2[:], C("w2hi", 128, 64), agg_hi[:],
                                 start=False, stop=True)
                h2v = sb.tile([64, 1], F16)
                with nc.allow_low_precision("fp16 h2"):
                    nc.vector.tensor_scalar(out=h2v[:], in0=p_h2[:],
                                            scalar1=(0.0 if b2_zero
                                                     else C("b2c", 64, 1)),
                                            scalar2=0.0,
                                            op0=ALU.add, op1=ALU.max)

                p_y = pss.tile([1, 64], F32, tag="pss")
                nc.tensor.matmul(p_y[:], h2v[:], C("wfct", 64, 64))
                y_sb = sb.tile([1, 64], F32)
                if b2_zero:
                    # h2 was aggregated with unnormalized exp weights;
                    # relu(x/d) = relu(x)/d for d > 0, so scale once here.
                    nc.vector.tensor_scalar_mul(out=y_sb[:], in0=p_y[:],
                                                scalar1=d2i[:])
                else:
                    nc.vector.tensor_scalar_mul(out=y_sb[:], in0=p_y[:],
                                                scalar1=1.0)
                if not bfc_zero:
                    nc.vector.tensor_add(out=y_sb[:], in0=y_sb[:],
                                         in1=C("bfcr", 1, 64))
                nc.sync.dma_start(out=out_d[:], in_=y_sb[:], single_packet=True)

    nc.compile()
    return nc


def kernel(**inputs):
    g = _prep(inputs)
    nc = _build_nc(g["n1"], g["D"], g["E1"], g["root_blk"], g["off"],
                   g["xall"].shape[1], g["wc"], g["b1_zero"], g["b2_zero"],
                   g["bfc_zero"])
    feed = {"xall": g["xall"]}
    res = run_bass_kernel_spmd(nc, [feed] * 8, core_ids=list(range(8)))
    return np.ascontiguousarray(res.results[0]["out"])
